# revision 4
# baseline (speedup 1.0000x reference)
"""Trainium2 Bass kernel for batched int8 matmul with fp32 dequant epilogue.

Problem: out[b, m, n] = alpha * sum_k a[b, m, k] * b[b, n, k]
  a: [64, 2048, 64] int8, b: [64, 2048, 64] int8, alpha: fp32 scalar
  out: [64, 2048, 2048] fp32

Sharding: batch dim across 8 NeuronCores (8 batches per core), no
communication. This problem is output-write bound: each core writes
128 MiB of fp32 to HBM (~375 us at ~358 GB/s), while inputs are only
2 MiB/core and compute is ~4.3 GMAC/core.

Per-core pipeline (per batch):
  1. SWDGE cast-DMA loads a[b]/b[b] int8 -> SBUF bf16 in a
     [128 (row-within-m-tile), 16 (m-tile), 64 (k)] layout. int8 values
     are exact in bf16.
  2. PE transposes each [128, 64] tile -> PSUM bf16 [64, 128]; DVE/ACT
     copy assembles aT/bT [64, 2048] bf16 in SBUF (k on partitions).
  3. bf16 matmuls aT_tile.T @ bT_slice -> fp32 PSUM [128, 512]; exact
     integer arithmetic (products <= 16129, sums < 2^24).
  4. DVE/ACT (alternating) scale by alpha, PSUM -> SBUF fp32.
  5. HWDGE DMA stores [128, 2048] fp32 tiles to HBM.
"""

import os
import numpy as np

M, N, K = 2048, 2048, 64
N_CORES = 8
B_TOTAL = 64
B_PER_CORE = B_TOTAL // N_CORES

_cache = {}


def _build(n_batches: int, alpha: float, m: int = M, n: int = N):
    import concourse.bacc as bacc
    import concourse.mybir as mybir
    import concourse.tile as tile
    from concourse.masks import make_identity

    MT = m // 128          # m-tiles
    NT = n // 128          # n-tiles
    NSLICE = 512
    NS = n // NSLICE       # n-slices per m-tile

    nc = bacc.Bacc("TRN2", target_bir_lowering=False, debug=False)
    a_dram = nc.dram_tensor("a", [n_batches, m, K], mybir.dt.int8, kind="ExternalInput")
    b_dram = nc.dram_tensor("b", [n_batches, n, K], mybir.dt.int8, kind="ExternalInput")
    out_dram = nc.dram_tensor(
        "out", [n_batches, m, n], mybir.dt.float32, kind="ExternalOutput"
    )

    with tile.TileContext(nc) as tc:
        with (
            tc.tile_pool(name="consts", bufs=1) as consts,
            tc.tile_pool(name="raw", bufs=2) as raw,
            tc.tile_pool(name="tp_psum", bufs=4, space="PSUM") as tp_psum,
            tc.tile_pool(name="mm_psum", bufs=4, space="PSUM") as mm_psum,
            tc.tile_pool(name="trans", bufs=2) as trans,
            tc.tile_pool(name="outp", bufs=4) as outp,
        ):
            ident = consts.tile([128, 128], mybir.dt.bfloat16)
            make_identity(nc, ident)

            eng_ctr = 0

            def copy_tp(dst, ps):
                nonlocal eng_ctr
                if eng_ctr % 2 == 0:
                    nc.vector.tensor_copy(out=dst, in_=ps)
                else:
                    nc.scalar.copy(out=dst, in_=ps)
                eng_ctr += 1

            def epilogue(dst, ps):
                nonlocal eng_ctr
                if eng_ctr % 2 == 0:
                    nc.vector.tensor_scalar_mul(dst, ps, alpha)
                else:
                    nc.scalar.mul(dst, ps, alpha)
                eng_ctr += 1

            for bb in range(n_batches):
                # A cast-load, fully contiguous per partition (128 x 1KiB
                # descriptors): partition p holds rows m = 16p + r, r in
                # [0, 16). The m-interleave is undone for free by the
                # store's partition -> DRAM-row mapping below.
                a_raw = raw.tile([128, MT, K], mybir.dt.bfloat16, tag="a_raw")
                nc.gpsimd.dma_start(
                    out=a_raw, in_=a_dram[bb].rearrange("(p r) k -> p r k", r=MT)
                )
                # B cast-load keeps the tiled layout (n must stay in true
                # order along the matmul free dim for contiguous stores).
                b_raw = raw.tile([128, NT, K], mybir.dt.bfloat16, tag="b_raw")
                nc.gpsimd.dma_start(
                    out=b_raw, in_=b_dram[bb].rearrange("(t p) k -> p t k", p=128)
                )

                aT = trans.tile([64, m], mybir.dt.bfloat16, tag="aT")
                bT = trans.tile([64, n], mybir.dt.bfloat16, tag="bT")
                # m-interleaved DRAM view for stores: row m = 16q + r
                out_view = out_dram[bb].rearrange("(q r) n2 -> r q n2", r=MT)

                def a_transpose(r):
                    ps = tp_psum.tile([64, 128], mybir.dt.bfloat16, tag="tp")
                    nc.tensor.transpose(ps, a_raw[:, r, :], ident)
                    copy_tp(aT[:, r * 128:(r + 1) * 128], ps)

                def b_transpose(t):
                    ps = tp_psum.tile([64, 128], mybir.dt.bfloat16, tag="tp")
                    nc.tensor.transpose(ps, b_raw[:, t, :], ident)
                    copy_tp(bT[:, t * 128:(t + 1) * 128], ps)

                def m_group(r, jit_b):
                    o_sb = outp.tile([128, n], mybir.dt.float32, tag="o_sb")
                    for s in range(NS):
                        if jit_b:
                            for t in range(s * NT // NS, (s + 1) * NT // NS):
                                b_transpose(t)
                        ps = mm_psum.tile([128, NSLICE], mybir.dt.float32, tag="mm")
                        nc.tensor.matmul(
                            ps,
                            aT[:, r * 128:(r + 1) * 128],
                            bT[:, s * NSLICE:(s + 1) * NSLICE],
                            start=True,
                            stop=True,
                        )
                        epilogue(o_sb[:, s * NSLICE:(s + 1) * NSLICE], ps)
                    nc.sync.dma_start(out=out_view[r], in_=o_sb)

                # m-group 0 first with just-in-time B transposes so the
                # first store issues as early as possible (head fill)
                a_transpose(0)
                m_group(0, jit_b=True)
                for r in range(1, MT):
                    a_transpose(r)
                    m_group(r, jit_b=False)

    nc.compile()
    return nc


def _get_nc(n_batches: int, alpha: float):
    key = (n_batches, float(alpha))
    if key not in _cache:
        _cache[key] = _build(n_batches, float(alpha))
    return _cache[key]


def kernel(a: np.ndarray, b: np.ndarray, alpha: np.ndarray) -> np.ndarray:
    from concourse.bass_utils import run_bass_kernel_spmd

    a = np.ascontiguousarray(np.asarray(a, dtype=np.int8))
    b = np.ascontiguousarray(np.asarray(b, dtype=np.int8))
    alpha_f = float(np.asarray(alpha, dtype=np.float32))

    nc = _get_nc(B_PER_CORE, alpha_f)

    in_maps = [
        {
            "a": a[c * B_PER_CORE:(c + 1) * B_PER_CORE],
            "b": b[c * B_PER_CORE:(c + 1) * B_PER_CORE],
        }
        for c in range(N_CORES)
    ]

    trace = bool(int(os.environ.get("BMM_TRACE", "0")))
    kwargs = {}
    if trace:
        kwargs["trace"] = True
        tdir = os.environ.get("BMM_TRACE_DIR")
        if tdir:
            os.makedirs(tdir, exist_ok=True)
            kwargs["tmpdir"] = tdir
    res = run_bass_kernel_spmd(nc, in_maps, core_ids=list(range(N_CORES)), **kwargs)
    if trace:
        kernel.last_exec_time_ns = res.exec_time_ns
        kernel.last_results = res
    out = np.concatenate([res.results[c]["out"] for c in range(N_CORES)], axis=0)
    return out


# revision 8
# speedup vs baseline: 1.0274x; 1.0274x over previous
"""Trainium2 Bass kernel for batched int8 matmul with fp32 dequant epilogue.

Problem: out[b, m, n] = alpha * sum_k a[b, m, k] * b[b, n, k]
  a: [64, 2048, 64] int8, b: [64, 2048, 64] int8, alpha: fp32 scalar
  out: [64, 2048, 2048] fp32

Sharding: batch dim across 8 NeuronCores (8 batches per core), no
communication. This problem is output-write bound: each core writes
128 MiB of fp32 to HBM (~375 us at ~358 GB/s), while inputs are only
2 MiB/core and compute is ~4.3 GMAC/core.

Per-core pipeline (per batch):
  1. SWDGE cast-DMA loads a[b]/b[b] int8 -> SBUF bf16 in a
     [128 (row-within-m-tile), 16 (m-tile), 64 (k)] layout. int8 values
     are exact in bf16.
  2. PE transposes each [128, 64] tile -> PSUM bf16 [64, 128]; DVE/ACT
     copy assembles aT/bT [64, 2048] bf16 in SBUF (k on partitions).
  3. bf16 matmuls aT_tile.T @ bT_slice -> fp32 PSUM [128, 512]; exact
     integer arithmetic (products <= 16129, sums < 2^24).
  4. DVE/ACT (alternating) scale by alpha, PSUM -> SBUF fp32.
  5. HWDGE DMA stores [128, 2048] fp32 tiles to HBM.
"""

import os
import numpy as np

M, N, K = 2048, 2048, 64
N_CORES = 8
B_TOTAL = 64
B_PER_CORE = B_TOTAL // N_CORES

_cache = {}


def _build(n_batches: int, alpha: float, m: int = M, n: int = N):
    import concourse.bacc as bacc
    import concourse.mybir as mybir
    import concourse.tile as tile
    from concourse.masks import make_identity

    MT = m // 128          # m-tiles
    NT = n // 128          # n-tiles
    NSLICE = 512
    NS = n // NSLICE       # n-slices per m-tile

    nc = bacc.Bacc("TRN2", target_bir_lowering=False, debug=False)
    a_dram = nc.dram_tensor("a", [n_batches, m, K], mybir.dt.int8, kind="ExternalInput")
    b_dram = nc.dram_tensor("b", [n_batches, n, K], mybir.dt.int8, kind="ExternalInput")
    out_dram = nc.dram_tensor(
        "out", [n_batches, m, n], mybir.dt.float32, kind="ExternalOutput"
    )

    with tile.TileContext(nc) as tc:
        with (
            tc.tile_pool(name="consts", bufs=1) as consts,
            tc.tile_pool(name="raw", bufs=2) as raw,
            tc.tile_pool(name="tp_psum", bufs=4, space="PSUM") as tp_psum,
            tc.tile_pool(name="mm_psum", bufs=4, space="PSUM") as mm_psum,
            tc.tile_pool(name="trans", bufs=2) as trans,
            tc.tile_pool(name="outp", bufs=4) as outp,
        ):
            ident = consts.tile([128, 128], mybir.dt.bfloat16)
            make_identity(nc, ident)

            eng_ctr = 0

            def copy_tp(dst, ps):
                nonlocal eng_ctr
                if eng_ctr % 2 == 0:
                    nc.vector.tensor_copy(out=dst, in_=ps)
                else:
                    nc.scalar.copy(out=dst, in_=ps)
                eng_ctr += 1

            def epilogue(dst, ps):
                nonlocal eng_ctr
                if eng_ctr % 2 == 0:
                    nc.vector.tensor_scalar_mul(dst, ps, alpha)
                else:
                    nc.scalar.mul(dst, ps, alpha)
                eng_ctr += 1

            for bb in range(n_batches):
                # Batch 0 (the pipeline head) loads in tiled order (64B
                # descriptors, but just-in-time transposes let the first
                # store issue early). Later batches load fully contiguous
                # (128 x 1KiB descriptors, ~6x less DMA descriptor tax) --
                # their transposes hide under the previous batch's stores.
                # Contiguous loads put row m = 16p + r on partition p; the
                # transpose of chunk r then yields columns p <-> m = 16p+r,
                # and writing through a stride-16 view restores true m/n
                # order in aT/bT so matmul slices and stores stay dense.
                contiguous = bb > 0
                a_raw = raw.tile([128, MT, K], mybir.dt.bfloat16, tag="a_raw")
                b_raw = raw.tile([128, NT, K], mybir.dt.bfloat16, tag="b_raw")
                if contiguous:
                    nc.gpsimd.dma_start(
                        out=a_raw, in_=a_dram[bb].rearrange("(p r) k -> p r k", r=MT)
                    )
                    nc.gpsimd.dma_start(
                        out=b_raw, in_=b_dram[bb].rearrange("(p r) k -> p r k", r=NT)
                    )
                else:
                    nc.gpsimd.dma_start(
                        out=a_raw, in_=a_dram[bb].rearrange("(t p) k -> p t k", p=128)
                    )
                    nc.gpsimd.dma_start(
                        out=b_raw, in_=b_dram[bb].rearrange("(t p) k -> p t k", p=128)
                    )

                aT = trans.tile([64, m], mybir.dt.bfloat16, tag="aT")
                bT = trans.tile([64, n], mybir.dt.bfloat16, tag="bT")
                aT_il = aT.rearrange("k (p r) -> k r p", r=MT)
                bT_il = bT.rearrange("k (p r) -> k r p", r=NT)

                def a_transpose(r):
                    ps = tp_psum.tile([64, 128], mybir.dt.bfloat16, tag="tp")
                    nc.tensor.transpose(ps, a_raw[:, r, :], ident)
                    copy_tp(aT_il[:, r, :] if contiguous else aT[:, r * 128:(r + 1) * 128], ps)

                def b_transpose(t):
                    ps = tp_psum.tile([64, 128], mybir.dt.bfloat16, tag="tp")
                    nc.tensor.transpose(ps, b_raw[:, t, :], ident)
                    copy_tp(bT_il[:, t, :] if contiguous else bT[:, t * 128:(t + 1) * 128], ps)

                def m_group(r, jit_b=False):
                    o_sb = outp.tile([128, n], mybir.dt.float32, tag="o_sb")
                    for s in range(NS):
                        if jit_b:
                            for t in range(s * NT // NS, (s + 1) * NT // NS):
                                b_transpose(t)
                        ps = mm_psum.tile([128, NSLICE], mybir.dt.float32, tag="mm")
                        nc.tensor.matmul(
                            ps,
                            aT[:, r * 128:(r + 1) * 128],
                            bT[:, s * NSLICE:(s + 1) * NSLICE],
                            start=True,
                            stop=True,
                        )
                        epilogue(o_sb[:, s * NSLICE:(s + 1) * NSLICE], ps)
                    nc.sync.dma_start(
                        out=out_dram[bb, r * 128:(r + 1) * 128, :], in_=o_sb
                    )

                if contiguous:
                    for r in range(MT):
                        a_transpose(r)
                    for t in range(NT):
                        b_transpose(t)
                    for r in range(MT):
                        m_group(r)
                else:
                    a_transpose(0)
                    m_group(0, jit_b=True)
                    for r in range(1, MT):
                        a_transpose(r)
                        m_group(r)

    nc.compile()
    return nc


def _get_nc(n_batches: int, alpha: float):
    key = (n_batches, float(alpha))
    if key not in _cache:
        _cache[key] = _build(n_batches, float(alpha))
    return _cache[key]


def kernel(a: np.ndarray, b: np.ndarray, alpha: np.ndarray) -> np.ndarray:
    from concourse.bass_utils import run_bass_kernel_spmd

    a = np.ascontiguousarray(np.asarray(a, dtype=np.int8))
    b = np.ascontiguousarray(np.asarray(b, dtype=np.int8))
    alpha_f = float(np.asarray(alpha, dtype=np.float32))

    nc = _get_nc(B_PER_CORE, alpha_f)

    in_maps = [
        {
            "a": a[c * B_PER_CORE:(c + 1) * B_PER_CORE],
            "b": b[c * B_PER_CORE:(c + 1) * B_PER_CORE],
        }
        for c in range(N_CORES)
    ]

    trace = bool(int(os.environ.get("BMM_TRACE", "0")))
    kwargs = {}
    if trace:
        kwargs["trace"] = True
        tdir = os.environ.get("BMM_TRACE_DIR")
        if tdir:
            import shutil

            shutil.rmtree(tdir, ignore_errors=True)
            os.makedirs(tdir, exist_ok=True)
            kwargs["tmpdir"] = tdir
    res = run_bass_kernel_spmd(nc, in_maps, core_ids=list(range(N_CORES)), **kwargs)
    if trace:
        kernel.last_exec_time_ns = res.exec_time_ns
        kernel.last_results = res
    out = np.concatenate([res.results[c]["out"] for c in range(N_CORES)], axis=0)
    return out
